# revision 10
# baseline (speedup 1.0000x reference)
"""Causal self-attention on 8 Trainium2 NeuronCores, head-sharded tensor parallel.

Contract: kernel(**inputs) takes the FULL unsharded inputs (x, W_qkv, b_qkv,
W_proj, b_proj) as numpy arrays and returns the FULL [B, T, C] float32 output.

Sharding: 16 heads / 8 cores = 2 heads per core. Each core computes qkv for
its heads, causal attention, and a partial output projection
(y_local @ W_proj[head_rows]); the host sums the 8 partials (the
tensor-parallel all-reduce, done at gather time) and adds b_proj.

Per-core kernel design (matmuls in bf16, fp32 accumulate):
- x is pre-transposed on host to xT [C, B*T] (bf16) so the contraction dim C
  is on SBUF partitions for the qkv matmuls.
- Q^T, K^T are produced in [d, t] layout (both heads packed into 128
  partitions, N=512 matmuls); V is computed directly in natural [t, d] layout
  (lhsT = xT tile), with a ones column appended per head ("V-aug") so the PV
  matmul also emits the softmax denominators l as psum row 64.
- Scores are computed transposed, S^T[j, i] = K Q^T, so exp(S^T) = P^T is
  born in the layout the PV matmul needs as rhs (no transposes of P).
- Softmax skips the max-subtraction: logits are ~N(0,1) by construction
  (1/sqrt(D) folded into Wq on host), max |logit| ~ 6, exp is safe in f32.
- Causal structure is exact at 128x128 tile granularity: above-diagonal
  tiles are never computed; diagonal tiles get an additive -87 mask via one
  merged N=256 matmul covering both heads.
- Y^T is normalized at eviction: the two l psum rows -> 1/l (DVE reciprocal,
  bf16) -> broadcast across partitions with a K=2 selector matmul -> the
  psum->SBUF eviction becomes a fused tensor_mul. The projection is then a
  single K=128 matmul per (t-block, 512-col chunk), evicted as a plain bf16
  copy alternating between DVE and GpSimd; the host sums bf16 partials.
- Emission is software-pipelined: S(jb) is emitted before PV(jb-1), and the
  previous chunk's projection matmuls are drip-fed between attention steps as
  PE filler, so the in-order PE queue never stalls on the Exp latency. A few
  warm-up matmuls at t=0 ramp the PE p-state while the x DMA lands.
"""
import sys

sys.path.insert(0, "/opt/trn_rl_repo")

import numpy as np
import ml_dtypes

import concourse.bacc as bacc
import concourse.bass as bass
import concourse.mybir as mybir
import concourse.tile as tile
from concourse import bass_utils

B, T, C, H, D = 2, 2048, 1024, 16, 64
NCORES = 8
HL = H // NCORES          # heads per core = 2
BT = B * T                # 4096
KT = C // 128             # 8 contraction tiles over C
NMC = BT // 1024          # 4 merged (1024-wide) column chunks over B*T
NTB = BT // 128           # 32 t-blocks of 128
NIC = T // 512            # 4 i-chunks per batch
BF16 = mybir.dt.bfloat16
F32 = mybir.dt.float32
AF = mybir.ActivationFunctionType
ALU = mybir.AluOpType
MASK_VAL = -87.0
N_WARMUP = 12

_compiled = {}


def _build():
    nc = bacc.Bacc("TRN2", target_bir_lowering=False, debug=False)

    xt_d = nc.dram_tensor("xt", [C, BT], BF16, kind="ExternalInput")
    wq_d = nc.dram_tensor("wq", [C, 128], BF16, kind="ExternalInput")
    wk_d = nc.dram_tensor("wk", [C, 128], BF16, kind="ExternalInput")
    wv_d = nc.dram_tensor("wv", [C, 128], BF16, kind="ExternalInput")
    wp_d = nc.dram_tensor("wp", [128, C], BF16, kind="ExternalInput")
    bq_d = nc.dram_tensor("bq", [128, 1], F32, kind="ExternalInput")
    bk_d = nc.dram_tensor("bk", [128, 1], F32, kind="ExternalInput")
    bvb_d = nc.dram_tensor("bvb", [128, 128], F32, kind="ExternalInput")
    mask2_d = nc.dram_tensor("mask2", [128, 256], BF16, kind="ExternalInput")
    idbf_d = nc.dram_tensor("idbf", [128, 128], BF16, kind="ExternalInput")
    out_d = nc.dram_tensor("out", [BT, C], BF16, kind="ExternalOutput")

    with tile.TileContext(nc) as tc:
        consts = tc.alloc_tile_pool(name="consts", bufs=1)
        bigbufs = tc.alloc_tile_pool(name="bigbufs", bufs=1)
        pts = tc.alloc_tile_pool(name="pts", bufs=3)
        lpool = tc.alloc_tile_pool(name="lpool", bufs=2)
        ostage = tc.alloc_tile_pool(name="ostage", bufs=4)
        psum = tc.alloc_tile_pool(name="psum", bufs=1, space="PSUM")

        def ps_s():
            return psum.tile([128, 2, 512], F32, tag="s", bufs=2, name="ps_s")

        # ---- tiny consts first: warm-up deps land in ~1us ----
        idbf_sb = consts.tile([128, 128], BF16)
        mask2_sb = consts.tile([128, 256], BF16)
        nc.sync.dma_start(out=idbf_sb[:], in_=idbf_d[:, :])
        nc.sync.dma_start(out=mask2_sb[:], in_=mask2_d[:, :])

        # ---- weights + x, in first-use order ----
        wq_sb = consts.tile([128, KT, 128], BF16)
        wk_sb = consts.tile([128, KT, 128], BF16)
        wv_sb = consts.tile([128, KT, 128], BF16)
        bq_sb = consts.tile([128, 1], F32)
        bk_sb = consts.tile([128, 1], F32)
        bvb_sb = consts.tile([128, 128], F32)
        wp_sb = consts.tile([128, C], BF16)
        xt_sb = bigbufs.tile([128, KT, BT], BF16)       # 8 MB

        nc.sync.dma_start(out=wq_sb[:], in_=wq_d.ap().rearrange("(k p) m -> p k m", p=128))
        for k in range(KT):
            nc.sync.dma_start(out=xt_sb[:, k, 0:1024],
                              in_=xt_d[k * 128:(k + 1) * 128, 0:1024])
        nc.sync.dma_start(out=bq_sb[:], in_=bq_d[:, :])
        nc.sync.dma_start(out=wk_sb[:], in_=wk_d.ap().rearrange("(k p) m -> p k m", p=128))
        nc.sync.dma_start(out=bk_sb[:], in_=bk_d[:, :])
        nc.sync.dma_start(
            out=xt_sb[:, :, 1024:2048],
            in_=xt_d[:, 1024:2048].rearrange("(k p) m -> p k m", p=128))
        nc.sync.dma_start(out=wv_sb[:], in_=wv_d.ap().rearrange("(k p) m -> p k m", p=128))
        nc.sync.dma_start(out=bvb_sb[:], in_=bvb_d[:, :])
        for g in (2, 3):
            nc.sync.dma_start(
                out=xt_sb[:, :, g * 1024:(g + 1) * 1024],
                in_=xt_d[:, g * 1024:(g + 1) * 1024].rearrange("(k p) m -> p k m", p=128))
        nc.sync.dma_start(out=wp_sb[:], in_=wp_d[:, :])

        # ---- persistent big buffers ----
        qT = bigbufs.tile([128, BT], BF16)              # [2h*64 d, t]
        kTt = bigbufs.tile([128, BT], BF16)
        v_sb = bigbufs.tile([128, NTB, 130], BF16)      # [t, tb, Vh0|1|Vh1|1]
        yt = bigbufs.tile([128, B, T], BF16)            # [2h*64 d, b, t] normalized

        nc.vector.memset(v_sb[:, :, 64:65], 1.0)
        nc.vector.memset(v_sb[:, :, 129:130], 1.0)

        # ---- PE warm-up: ramp the p-state while the x DMA lands ----
        wm = ps_s()
        for i in range(N_WARMUP):
            nc.tensor.matmul(wm[:, i % 2, (i % 4 // 2) * 128:(i % 4 // 2) * 128 + 128],
                             lhsT=idbf_sb[:], rhs=idbf_sb[:], start=True, stop=True)

        # fillers: deque of closures emitting one projection chunk each,
        # drip-fed into the next chunk's attention loop as PE filler.
        # pending_tail: the previous chunk's deferred normalize+evict.
        fillers = []
        pending_tail = [None]

        def make_proj(b, ci):
            state = {}

            def chunk(tb4, cc):
                def emit():
                    tgp = ci * 512 + tb4 * 128
                    if cc == 0:
                        state[tb4] = ostage.tile([128, 1024], BF16, tag="ot",
                                                 bufs=4, name="ot")
                    ot = state[tb4]
                    pj = psum.tile([128, 512], F32, tag="pj", bufs=2, name="pj")
                    nc.tensor.matmul(pj[:], lhsT=yt[:, b, tgp:tgp + 128],
                                     rhs=wp_sb[:, cc * 512:(cc + 1) * 512],
                                     start=True, stop=True)
                    nc.vector.tensor_copy(ot[:, cc * 512:(cc + 1) * 512], pj[:])
                    if cc == 1:
                        nc.sync.dma_start(
                            out=out_d[b * T + tgp:b * T + tgp + 128, :], in_=ot[:])
                return emit
            return [chunk(tb4, cc) for tb4 in range(4) for cc in range(2)]

        # ---- per batch: qkv for that batch, then attention ----
        for b in range(B):
            # Q^T / K^T for this batch's chunks. The previous batch's
            # deferred tail MUST flush before the V phase reuses the y-tag
            # psum ring, and its projection drains here as PE filler.
            qk_chunk = 0
            for w_sb, b_sb, dst in ((wq_sb, bq_sb, qT), (wk_sb, bk_sb, kTt)):
                for mc in (2 * b, 2 * b + 1):
                    ps = ps_s()
                    for k in range(KT):
                        for half in range(2):
                            nc.tensor.matmul(
                                ps[:, half, :], lhsT=w_sb[:, k, :],
                                rhs=xt_sb[:, k, mc * 1024 + half * 512:mc * 1024 + (half + 1) * 512],
                                start=(k == 0), stop=(k == KT - 1))
                    nc.vector.tensor_scalar_add(
                        dst[:, bass.ts(mc, 1024)],
                        ps.rearrange("p h x -> p (h x)"), b_sb[:])
                    if qk_chunk == 0 and pending_tail[0] is not None:
                        pending_tail[0]()
                        pending_tail[0] = None
                    if qk_chunk >= 1:
                        for _ in range(3):
                            if fillers:
                                fillers.pop(0)()
                    qk_chunk += 1

            # V in natural [t, d] layout: lhsT = xT tile, rhs = Wv
            for tb in range(16 * b, 16 * (b + 1)):
                pv = psum.tile([128, 128], F32, tag="y", bufs=2)
                for k in range(KT):
                    nc.tensor.matmul(pv[:], lhsT=xt_sb[:, k, bass.ts(tb, 128)],
                                     rhs=wv_sb[:, k, :], start=(k == 0), stop=(k == KT - 1))
                nc.vector.tensor_add(v_sb[:, tb, 0:64], pv[:, 0:64], bvb_sb[:, 0:64])
                nc.vector.tensor_add(v_sb[:, tb, 65:129], pv[:, 64:128], bvb_sb[:, 64:128])

            for ci in (range(NIC) if b == 0 else range(NIC - 1, -1, -1)):
                y0 = psum.tile([65, 512], F32, tag="y", bufs=2, name="y0")
                y1 = psum.tile([65, 512], F32, tag="y", bufs=2, name="y1")
                njb = 4 * (ci + 1)
                tg = b * T + ci * 512

                def emit_pv(jb, lo, ptb, last):
                    vt = b * 16 + jb
                    nc.tensor.matmul(y0[:, lo:512], lhsT=v_sb[:, vt, 0:65],
                                     rhs=ptb[:, 0, lo:512],
                                     start=(jb == 0), stop=last)
                    nc.tensor.matmul(y1[:, lo:512], lhsT=v_sb[:, vt, 65:130],
                                     rhs=ptb[:, 1, lo:512],
                                     start=(jb == 0), stop=last)

                pvq = []
                for jb in range(njb):
                    sb = max(0, jb - 4 * ci)
                    lo = sb * 128
                    jg = b * T + jb * 128
                    s2 = ps_s()
                    diag = jb >= 4 * ci
                    nc.tensor.matmul(s2[:, 0, lo:512], lhsT=kTt[0:64, jg:jg + 128],
                                     rhs=qT[0:64, tg + lo:tg + 512], start=True,
                                     stop=not diag)
                    nc.tensor.matmul(s2[:, 1, lo:512], lhsT=kTt[64:128, jg:jg + 128],
                                     rhs=qT[64:128, tg + lo:tg + 512], start=True,
                                     stop=not diag)
                    if diag:  # add the causal mask onto the diagonal 128 cols
                        nc.tensor.matmul(s2[:, :, lo:lo + 128], lhsT=idbf_sb[:],
                                         rhs=mask2_sb[:], start=False, stop=True)
                    ptb = pts.tile([128, 2, 512], BF16, tag="pt", bufs=5)
                    nc.scalar.activation(ptb[:, :, lo:512], s2[:, :, lo:512], AF.Exp)
                    pvq.append((jb, lo, ptb))
                    if len(pvq) > 3:  # PV lags S by 3: dep slack for the PE queue
                        emit_pv(*pvq.pop(0), last=False)
                    if jb == 1 and pending_tail[0] is not None:
                        pending_tail[0]()
                        pending_tail[0] = None
                    if jb >= 3:
                        for _ in range(2):
                            if fillers:
                                fillers.pop(0)()
                while pvq:
                    emit_pv(*pvq.pop(0), last=(len(pvq) == 0))
                if pending_tail[0] is not None:
                    pending_tail[0]()
                    pending_tail[0] = None
                while fillers:
                    fillers.pop(0)()

                # 1/l rows via DVE; y psums stay live until the muls below
                lrA = lpool.tile([1, 512], F32, tag="lr", bufs=4, name="lrA")
                lrB = lpool.tile([1, 512], F32, tag="lr", bufs=4, name="lrB")
                nc.vector.reciprocal(lrA[:], y0[64:65, :])
                nc.vector.reciprocal(lrB[:], y1[64:65, :])

                def make_tail(b, ci, y0, y1, lrA, lrB):
                    def emit():
                        bc0 = lpool.tile([64, 512], F32, tag="bc", bufs=4,
                                         name="bc0")
                        bc1 = lpool.tile([64, 512], F32, tag="bc", bufs=4,
                                         name="bc1")
                        nc.gpsimd.partition_broadcast(bc0[:], lrA[:])
                        nc.gpsimd.partition_broadcast(bc1[:], lrB[:])
                        cs = slice(ci * 512, (ci + 1) * 512)
                        nc.vector.tensor_mul(yt[0:64, b, cs], y0[0:64, :], bc0[:])
                        nc.vector.tensor_mul(yt[64:128, b, cs], y1[0:64, :], bc1[:])
                    return emit

                pending_tail[0] = make_tail(b, ci, y0, y1, lrA, lrB)
                fillers.extend(make_proj(b, ci))

        if pending_tail[0] is not None:
            pending_tail[0]()
            pending_tail[0] = None
        while fillers:
            fillers.pop(0)()

        for pool in (psum, ostage, lpool, pts, bigbufs, consts):
            pool.release()

    nc.compile()
    return nc


def _prep_inputs(x, W_qkv, b_qkv, W_proj, b_proj):
    """Host-side sharding/layout prep. Returns per-core in_maps."""
    bf16 = ml_dtypes.bfloat16
    x2 = np.ascontiguousarray(x.reshape(BT, C).T).astype(bf16)  # [C, B*T]
    scale = 1.0 / np.sqrt(D)

    jj, ii = np.meshgrid(np.arange(128), np.arange(128), indexing="ij")
    maskbf = np.where(jj <= ii, 0.0, MASK_VAL).astype(bf16)
    mask2 = np.concatenate([maskbf, maskbf], axis=1)  # [128, 256]
    idbf = np.eye(128).astype(bf16)
    in_maps = []
    for core in range(NCORES):
        s = slice(128 * core, 128 * (core + 1))
        wq = (W_qkv[:, 0:C][:, s] * scale).astype(bf16)
        wk = W_qkv[:, C:2 * C][:, s].astype(bf16)
        wv = W_qkv[:, 2 * C:3 * C][:, s].astype(bf16)
        wp = W_proj[s, :].astype(bf16)
        bq = (b_qkv[0:C][s] * scale).astype(np.float32).reshape(128, 1)
        bk = b_qkv[C:2 * C][s].astype(np.float32).reshape(128, 1)
        bv = b_qkv[2 * C:3 * C][s].astype(np.float32)
        bvb = np.broadcast_to(bv, (128, 128)).copy()
        in_maps.append({
            "xt": x2, "wq": wq, "wk": wk, "wv": wv, "wp": wp,
            "bq": bq, "bk": bk, "bvb": bvb, "mask2": mask2, "idbf": idbf,
        })
    return in_maps


def kernel(x, W_qkv, b_qkv, W_proj, b_proj, _trace=False, _return_raw=False,
           _tmpdir=None):
    x = np.asarray(x, dtype=np.float32)
    W_qkv = np.asarray(W_qkv, dtype=np.float32)
    b_qkv = np.asarray(b_qkv, dtype=np.float32)
    W_proj = np.asarray(W_proj, dtype=np.float32)
    b_proj = np.asarray(b_proj, dtype=np.float32)

    if "nc" not in _compiled:
        _compiled["nc"] = _build()
    nc = _compiled["nc"]

    in_maps = _prep_inputs(x, W_qkv, b_qkv, W_proj, b_proj)
    kw = {}
    if _tmpdir is not None:
        kw["tmpdir"] = _tmpdir
    res = bass_utils.run_bass_kernel_spmd(
        nc, in_maps, core_ids=list(range(NCORES)), trace=_trace, **kw)

    acc = np.zeros((BT, C), dtype=np.float32)
    for core in range(NCORES):
        acc += res.results[core]["out"].astype(np.float32)
    acc += b_proj[None, :]
    out = acc.reshape(B, T, C)
    if _return_raw:
        return out, res
    return out


# revision 12
# speedup vs baseline: 1.1908x; 1.1908x over previous
"""Causal self-attention on 8 Trainium2 NeuronCores, head-sharded tensor parallel.

Contract: kernel(**inputs) takes the FULL unsharded inputs (x, W_qkv, b_qkv,
W_proj, b_proj) as numpy arrays and returns the FULL [B, T, C] float32 output.

Sharding: 16 heads / 8 cores = 2 heads per core. Each core computes qkv for
its heads, causal attention, and a partial output projection
(y_local @ W_proj[head_rows]); the host sums the 8 partials (the
tensor-parallel all-reduce, done at gather time) and adds b_proj.

Per-core kernel design (matmuls in bf16, fp32 accumulate):
- x is pre-transposed on host to xT [C, B*T] (bf16) so the contraction dim C
  is on SBUF partitions for the qkv matmuls.
- Q^T, K^T are produced in [d, t] layout (both heads packed into 128
  partitions, N=512 matmuls); V is computed directly in natural [t, d] layout
  (lhsT = xT tile), with a ones column appended per head ("V-aug") so the PV
  matmul also emits the softmax denominators l as psum row 64.
- Scores are computed transposed, S^T[j, i] = K Q^T, so exp(S^T) = P^T is
  born in the layout the PV matmul needs as rhs (no transposes of P).
- Softmax skips the max-subtraction: logits are ~N(0,1) by construction
  (1/sqrt(D) folded into Wq on host), max |logit| ~ 6, exp is safe in f32.
- Causal structure is exact at 128x128 tile granularity: above-diagonal
  tiles are never computed; diagonal tiles get an additive -87 mask via one
  merged N=256 matmul covering both heads.
- Y^T is normalized at eviction: the two l psum rows -> 1/l (DVE reciprocal,
  bf16) -> broadcast across partitions with a K=2 selector matmul -> the
  psum->SBUF eviction becomes a fused tensor_mul. The projection is then a
  single K=128 matmul per (t-block, 512-col chunk), evicted as a plain bf16
  copy alternating between DVE and GpSimd; the host sums bf16 partials.
- Emission is software-pipelined: S(jb) is emitted before PV(jb-1), and the
  previous chunk's projection matmuls are drip-fed between attention steps as
  PE filler, so the in-order PE queue never stalls on the Exp latency. A few
  warm-up matmuls at t=0 ramp the PE p-state while the x DMA lands.
"""
import sys

sys.path.insert(0, "/opt/trn_rl_repo")

import numpy as np
import ml_dtypes

import concourse.bacc as bacc
import concourse.bass as bass
import concourse.mybir as mybir
import concourse.tile as tile
from concourse import bass_utils

B, T, C, H, D = 2, 2048, 1024, 16, 64
NCORES = 8
HL = H // NCORES          # heads per core = 2
BT = B * T                # 4096
KT = C // 128             # 8 contraction tiles over C
NMC = BT // 1024          # 4 merged (1024-wide) column chunks over B*T
NTB = BT // 128           # 32 t-blocks of 128
NIC = T // 512            # 4 i-chunks per batch
BF16 = mybir.dt.bfloat16
F32 = mybir.dt.float32
AF = mybir.ActivationFunctionType
ALU = mybir.AluOpType
MASK_VAL = -87.0
N_WARMUP = 12

_compiled = {}


def _build():
    nc = bacc.Bacc("TRN2", target_bir_lowering=False, debug=False)

    xt_d = nc.dram_tensor("xt", [C, BT], BF16, kind="ExternalInput")
    wq_d = nc.dram_tensor("wq", [C, 128], BF16, kind="ExternalInput")
    wk_d = nc.dram_tensor("wk", [C, 128], BF16, kind="ExternalInput")
    wv_d = nc.dram_tensor("wv", [C, 128], BF16, kind="ExternalInput")
    wp_d = nc.dram_tensor("wp", [128, C], BF16, kind="ExternalInput")
    bq_d = nc.dram_tensor("bq", [128, 1], F32, kind="ExternalInput")
    bk_d = nc.dram_tensor("bk", [128, 1], F32, kind="ExternalInput")
    bvb_d = nc.dram_tensor("bvb", [128, 128], F32, kind="ExternalInput")
    mask2_d = nc.dram_tensor("mask2", [128, 256], BF16, kind="ExternalInput")
    idbf_d = nc.dram_tensor("idbf", [128, 128], BF16, kind="ExternalInput")
    out_d = nc.dram_tensor("out", [BT, C], BF16, kind="ExternalOutput")

    with tile.TileContext(nc) as tc:
        consts = tc.alloc_tile_pool(name="consts", bufs=1)
        bigbufs = tc.alloc_tile_pool(name="bigbufs", bufs=1)
        pts = tc.alloc_tile_pool(name="pts", bufs=3)
        lpool = tc.alloc_tile_pool(name="lpool", bufs=2)
        ostage = tc.alloc_tile_pool(name="ostage", bufs=4)
        psum = tc.alloc_tile_pool(name="psum", bufs=1, space="PSUM")

        def ps_s():
            return psum.tile([128, 2, 512], F32, tag="s", bufs=2, name="ps_s")

        # ---- tiny consts first: warm-up deps land in ~1us ----
        idbf_sb = consts.tile([128, 128], BF16)
        mask2_sb = consts.tile([128, 256], BF16)
        nc.sync.dma_start(out=idbf_sb[:], in_=idbf_d[:, :])
        nc.sync.dma_start(out=mask2_sb[:], in_=mask2_d[:, :])

        # ---- weights + x, in first-use order ----
        wq_sb = consts.tile([128, KT, 128], BF16)
        wk_sb = consts.tile([128, KT, 128], BF16)
        wv_sb = consts.tile([128, KT, 128], BF16)
        bq_sb = consts.tile([128, 1], F32)
        bk_sb = consts.tile([128, 1], F32)
        bvb_sb = consts.tile([128, 128], F32)
        wp_sb = consts.tile([128, C], BF16)
        xt_sb = bigbufs.tile([128, KT, BT], BF16)       # 8 MB

        nc.sync.dma_start(out=wq_sb[:], in_=wq_d.ap().rearrange("(k p) m -> p k m", p=128))
        for k in range(KT):
            nc.sync.dma_start(out=xt_sb[:, k, 0:1024],
                              in_=xt_d[k * 128:(k + 1) * 128, 0:1024])
        nc.sync.dma_start(out=bq_sb[:], in_=bq_d[:, :])
        nc.sync.dma_start(out=wk_sb[:], in_=wk_d.ap().rearrange("(k p) m -> p k m", p=128))
        nc.sync.dma_start(out=bk_sb[:], in_=bk_d[:, :])
        nc.sync.dma_start(
            out=xt_sb[:, :, 1024:2048],
            in_=xt_d[:, 1024:2048].rearrange("(k p) m -> p k m", p=128))
        nc.sync.dma_start(out=wv_sb[:], in_=wv_d.ap().rearrange("(k p) m -> p k m", p=128))
        nc.sync.dma_start(out=bvb_sb[:], in_=bvb_d[:, :])
        for g in (2, 3):
            nc.sync.dma_start(
                out=xt_sb[:, :, g * 1024:(g + 1) * 1024],
                in_=xt_d[:, g * 1024:(g + 1) * 1024].rearrange("(k p) m -> p k m", p=128))
        nc.sync.dma_start(out=wp_sb[:], in_=wp_d[:, :])

        # ---- persistent big buffers ----
        qT = bigbufs.tile([128, BT], BF16)              # [2h*64 d, t]
        kTt = bigbufs.tile([128, BT], BF16)
        v_sb = bigbufs.tile([128, NTB, 130], BF16)      # [t, tb, Vh0|1|Vh1|1]
        yt = bigbufs.tile([128, B, T], BF16)            # [2h*64 d, b, t] normalized

        nc.vector.memset(v_sb[:, :, 64:65], 1.0)
        nc.vector.memset(v_sb[:, :, 129:130], 1.0)

        # ---- PE warm-up: ramp the p-state while the x DMA lands ----
        wm = ps_s()
        for i in range(N_WARMUP):
            nc.tensor.matmul(wm[:, i % 2, (i % 4 // 2) * 128:(i % 4 // 2) * 128 + 128],
                             lhsT=idbf_sb[:], rhs=idbf_sb[:], start=True, stop=True)

        # fillers: deque of closures emitting one projection chunk each,
        # drip-fed into the next chunk's attention loop as PE filler.
        # pending_tail: the previous chunk's deferred normalize+evict.
        fillers = []
        pending_tail = [None]

        def make_proj(b, ci):
            state = {}

            def chunk(tb4, cc):
                def emit():
                    tgp = ci * 512 + tb4 * 128
                    if cc == 0:
                        state[tb4] = ostage.tile([128, 1024], BF16, tag="ot",
                                                 bufs=4, name="ot")
                    ot = state[tb4]
                    pj = psum.tile([128, 512], F32, tag="pj", bufs=2, name="pj")
                    nc.tensor.matmul(pj[:], lhsT=yt[:, b, tgp:tgp + 128],
                                     rhs=wp_sb[:, cc * 512:(cc + 1) * 512],
                                     start=True, stop=True)
                    nc.vector.tensor_copy(ot[:, cc * 512:(cc + 1) * 512], pj[:])
                    if cc == 1:
                        nc.sync.dma_start(
                            out=out_d[b * T + tgp:b * T + tgp + 128, :], in_=ot[:])
                return emit
            return [chunk(tb4, cc) for tb4 in range(4) for cc in range(2)]

        # ---- per batch: qkv for that batch, then attention ----
        for b in range(B):
            # Q^T / K^T for this batch's chunks. The previous batch's
            # deferred tail MUST flush before the V phase reuses the y-tag
            # psum ring, and its projection drains here as PE filler.
            qk_chunk = 0
            for w_sb, b_sb, dst in ((wq_sb, bq_sb, qT), (wk_sb, bk_sb, kTt)):
                for mc in (2 * b, 2 * b + 1):
                    ps = ps_s()
                    for k in range(KT):
                        for half in range(2):
                            nc.tensor.matmul(
                                ps[:, half, :], lhsT=w_sb[:, k, :],
                                rhs=xt_sb[:, k, mc * 1024 + half * 512:mc * 1024 + (half + 1) * 512],
                                start=(k == 0), stop=(k == KT - 1))
                    nc.vector.tensor_scalar_add(
                        dst[:, bass.ts(mc, 1024)],
                        ps.rearrange("p h x -> p (h x)"), b_sb[:])
                    if qk_chunk == 0 and pending_tail[0] is not None:
                        pending_tail[0]()
                        pending_tail[0] = None
                    if qk_chunk >= 1:
                        for _ in range(3):
                            if fillers:
                                fillers.pop(0)()
                    qk_chunk += 1

            # V in natural [t, d] layout: lhsT = xT tile, rhs = Wv
            for tb in range(16 * b, 16 * (b + 1)):
                pv = psum.tile([128, 128], F32, tag="y", bufs=2)
                for k in range(KT):
                    nc.tensor.matmul(pv[:], lhsT=xt_sb[:, k, bass.ts(tb, 128)],
                                     rhs=wv_sb[:, k, :], start=(k == 0), stop=(k == KT - 1))
                nc.vector.tensor_add(v_sb[:, tb, 0:64], pv[:, 0:64], bvb_sb[:, 0:64])
                nc.vector.tensor_add(v_sb[:, tb, 65:129], pv[:, 64:128], bvb_sb[:, 64:128])

            for ci in (range(NIC) if b == 0 else range(NIC - 1, -1, -1)):
                y0 = psum.tile([65, 512], F32, tag="y", bufs=2, name="y0")
                y1 = psum.tile([65, 512], F32, tag="y", bufs=2, name="y1")
                njb = 4 * (ci + 1)
                tg = b * T + ci * 512

                def emit_pv(jb, lo, ptb, last):
                    vt = b * 16 + jb
                    nc.tensor.matmul(y0[:, lo:512], lhsT=v_sb[:, vt, 0:65],
                                     rhs=ptb[:, 0, lo:512],
                                     start=(jb == 0), stop=last)
                    nc.tensor.matmul(y1[:, lo:512], lhsT=v_sb[:, vt, 65:130],
                                     rhs=ptb[:, 1, lo:512],
                                     start=(jb == 0), stop=last)

                pvq = []
                for jb in range(njb):
                    sb = max(0, jb - 4 * ci)
                    lo = sb * 128
                    jg = b * T + jb * 128
                    s2 = ps_s()
                    diag = jb >= 4 * ci
                    nc.tensor.matmul(s2[:, 0, lo:512], lhsT=kTt[0:64, jg:jg + 128],
                                     rhs=qT[0:64, tg + lo:tg + 512], start=True,
                                     stop=not diag)
                    nc.tensor.matmul(s2[:, 1, lo:512], lhsT=kTt[64:128, jg:jg + 128],
                                     rhs=qT[64:128, tg + lo:tg + 512], start=True,
                                     stop=not diag)
                    if diag:  # add the causal mask onto the diagonal 128 cols
                        nc.tensor.matmul(s2[:, :, lo:lo + 128], lhsT=idbf_sb[:],
                                         rhs=mask2_sb[:], start=False, stop=True)
                    ptb = pts.tile([128, 2, 512], BF16, tag="pt", bufs=5)
                    nc.scalar.activation(ptb[:, :, lo:512], s2[:, :, lo:512], AF.Exp)
                    pvq.append((jb, lo, ptb))
                    if len(pvq) > 4:  # PV lags S by 4: dep slack for the PE queue
                        emit_pv(*pvq.pop(0), last=False)
                    if jb == 1 and pending_tail[0] is not None:
                        pending_tail[0]()
                        pending_tail[0] = None
                    if jb >= 3:
                        for _ in range(2):
                            if fillers:
                                fillers.pop(0)()
                while pvq:
                    emit_pv(*pvq.pop(0), last=(len(pvq) == 0))
                if pending_tail[0] is not None:
                    pending_tail[0]()
                    pending_tail[0] = None
                while fillers:
                    fillers.pop(0)()

                # l rows -> SBUF via Act (frees nothing but decouples the
                # custom-DVE recip from psum reads), then fast 1/l on DVE
                lw0 = lpool.tile([1, 512], F32, tag="lw", bufs=4, name="lw0")
                lw1 = lpool.tile([1, 512], F32, tag="lw", bufs=4, name="lw1")
                nc.scalar.copy(lw0[:], y0[64:65, :])
                nc.scalar.copy(lw1[:], y1[64:65, :])
                lrA = lpool.tile([1, 512], F32, tag="lr", bufs=4, name="lrA")
                lrB = lpool.tile([1, 512], F32, tag="lr", bufs=4, name="lrB")
                nc.vector.reciprocal_approx_fast(lrA[:], lw0[:])
                nc.vector.reciprocal_approx_fast(lrB[:], lw1[:])

                def make_tail(b, ci, y0, y1, lrA, lrB):
                    def emit():
                        bc0 = lpool.tile([64, 512], F32, tag="bc", bufs=4,
                                         name="bc0")
                        bc1 = lpool.tile([64, 512], F32, tag="bc", bufs=4,
                                         name="bc1")
                        nc.gpsimd.partition_broadcast(bc0[:], lrA[:])
                        nc.gpsimd.partition_broadcast(bc1[:], lrB[:])
                        cs = slice(ci * 512, (ci + 1) * 512)
                        nc.vector.tensor_mul(yt[0:64, b, cs], y0[0:64, :], bc0[:])
                        nc.vector.tensor_mul(yt[64:128, b, cs], y1[0:64, :], bc1[:])
                    return emit

                pending_tail[0] = make_tail(b, ci, y0, y1, lrA, lrB)
                fillers.extend(make_proj(b, ci))

        if pending_tail[0] is not None:
            pending_tail[0]()
            pending_tail[0] = None
        while fillers:
            fillers.pop(0)()

        for pool in (psum, ostage, lpool, pts, bigbufs, consts):
            pool.release()

    nc.compile()
    return nc


def _prep_inputs(x, W_qkv, b_qkv, W_proj, b_proj):
    """Host-side sharding/layout prep. Returns per-core in_maps."""
    bf16 = ml_dtypes.bfloat16
    x2 = np.ascontiguousarray(x.reshape(BT, C).T).astype(bf16)  # [C, B*T]
    scale = 1.0 / np.sqrt(D)

    jj, ii = np.meshgrid(np.arange(128), np.arange(128), indexing="ij")
    maskbf = np.where(jj <= ii, 0.0, MASK_VAL).astype(bf16)
    mask2 = np.concatenate([maskbf, maskbf], axis=1)  # [128, 256]
    idbf = np.eye(128).astype(bf16)
    in_maps = []
    for core in range(NCORES):
        s = slice(128 * core, 128 * (core + 1))
        wq = (W_qkv[:, 0:C][:, s] * scale).astype(bf16)
        wk = W_qkv[:, C:2 * C][:, s].astype(bf16)
        wv = W_qkv[:, 2 * C:3 * C][:, s].astype(bf16)
        wp = W_proj[s, :].astype(bf16)
        bq = (b_qkv[0:C][s] * scale).astype(np.float32).reshape(128, 1)
        bk = b_qkv[C:2 * C][s].astype(np.float32).reshape(128, 1)
        bv = b_qkv[2 * C:3 * C][s].astype(np.float32)
        bvb = np.broadcast_to(bv, (128, 128)).copy()
        in_maps.append({
            "xt": x2, "wq": wq, "wk": wk, "wv": wv, "wp": wp,
            "bq": bq, "bk": bk, "bvb": bvb, "mask2": mask2, "idbf": idbf,
        })
    return in_maps


def kernel(x, W_qkv, b_qkv, W_proj, b_proj, _trace=False, _return_raw=False,
           _tmpdir=None):
    x = np.asarray(x, dtype=np.float32)
    W_qkv = np.asarray(W_qkv, dtype=np.float32)
    b_qkv = np.asarray(b_qkv, dtype=np.float32)
    W_proj = np.asarray(W_proj, dtype=np.float32)
    b_proj = np.asarray(b_proj, dtype=np.float32)

    if "nc" not in _compiled:
        _compiled["nc"] = _build()
    nc = _compiled["nc"]

    in_maps = _prep_inputs(x, W_qkv, b_qkv, W_proj, b_proj)
    kw = {}
    if _tmpdir is not None:
        kw["tmpdir"] = _tmpdir
    res = bass_utils.run_bass_kernel_spmd(
        nc, in_maps, core_ids=list(range(NCORES)), trace=_trace, **kw)

    acc = np.zeros((BT, C), dtype=np.float32)
    for core in range(NCORES):
        acc += res.results[core]["out"].astype(np.float32)
    acc += b_proj[None, :]
    out = acc.reshape(B, T, C)
    if _return_raw:
        return out, res
    return out
